# revision 21
# baseline (speedup 1.0000x reference)
"""LocallyConnected1d (untied-weight conv1d) on 8 Trainium2 NeuronCores.

Problem (hardcoded):
    x:      (B=128, C=64, L=1024) f32
    weight: (O=64, C=64, P=1024, K=7) f32   (untied per output position)
    bias:   (O=64, P=1024) f32
    out:    (B=128, O=64, P=1024) f32
    out[b,o,p] = sum_{c,k} xpad[b,c,p+k] * w[o,c,p,k] + bias[o,p]  (pad=3)

Sharding: sequence-parallel over P — core m owns positions [128m, 128m+128).
Each weight element is used exactly once, so this minimizes HBM traffic
(weight shard dominates); x, weight and the output are cast to bf16
(PSUM accumulation stays fp32), halving the dominant DMA traffic.

Per-core algorithm (pair scheme): input columns are processed in PAIRS
(j=2t, 2t+1) stacked into the full 128-partition contraction dim
[(h, c) = 2*64]. Pair t's stationary operand is the x pair-column
[128, b=128]; for each PSUM bank g (8 output positions), the 7 pairs
t=4g..4g+6 contribute moving blocks of untied weights [128, width*64]
(width = overlap of the pair's 8-position span with the bank, summing to
32 position-instances = 2048 moving columns per bank — half the moving
columns of a 64-row scheme, which matters because the HAM clock gate
keeps the PE at 1.2GHz in this DMA-bound regime: PE work must fit under
the DMA time even when cold). Positions covered by only one column of
the pair get 64 zero rows in the weight pack (~12% pad bytes). Bias is
added by opening each bank's accumulation group with a K=1 ones-x-bias
matmul that writes the full bank.
"""

import numpy as np
import ml_dtypes

BF16 = ml_dtypes.bfloat16

B = 128
C = 64
O = 64
L = 1024
KW = 7
PAD = 3
NCORES = 8
PC = L // NCORES          # positions per core = 128
NJ = PC + 2 * PAD         # input columns per core incl halo = 134
NT = NJ // 2              # column pairs = 67
NBANK = PC // 8           # psum banks of 8 positions = 16
BANKW = 8 * O             # psum bank free width = 512 f32


def _bank_pairs(g):
    """Pairs contributing to bank g: (t, lo, hi) position ranges."""
    out = []
    for t in range(NT):
        lo = max(8 * g, 2 * t - (KW - 1), 0)
        hi = min(8 * g + 7, 2 * t + 1, PC - 1)
        if lo <= hi:
            out.append((t, lo, hi))
    return out


BANKS = [_bank_pairs(g) for g in range(NBANK)]
WOFF = {}                 # weight-pack column offset of block (g, t)
_off = 0
for _g in range(NBANK):
    for _t, _lo, _hi in BANKS[_g]:
        WOFF[(_g, _t)] = _off
        _off += (_hi - _lo + 1) * O
WCOLS = _off              # 32768
BANK_C0 = [WOFF[(g, BANKS[g][0][0])] for g in range(NBANK)] + [WCOLS]

# x DMA chunks in pairs; weight DMA chunks in banks (small first chunk so
# compute starts early, ~1MB mid-kernel, small last chunk for a short tail).
XCHUNKS = [(0, 7), (7, 14), (21, 16), (37, 15), (52, 15)]
WCHUNKS = []
_g = 0
for _n in [1] + [2] * 7 + [1]:
    WCHUNKS.append((_g, _n))
    _g += _n
assert _g == NBANK


def _xchunk_of(t):
    for i, (t0, n) in enumerate(XCHUNKS):
        if t0 <= t < t0 + n:
            return i, t0
    raise AssertionError


def _wchunk_of(g):
    for i, (g0, n) in enumerate(WCHUNKS):
        if g0 <= g < g0 + n:
            return i, g0
    raise AssertionError


def _pack_inputs(x, weight, bias):
    """Host-side relayout into DMA/matmul-friendly per-core arrays (bf16)."""
    xp = np.zeros((B, C, L + 2 * PAD), np.float32)
    xp[:, :, PAD:PAD + L] = x
    # (C, 1030, B): column-major access per (c, j)
    xt = np.ascontiguousarray(xp.transpose(1, 2, 0)).astype(BF16)

    xpacks = []
    for m in range(NCORES):
        s = xt[:, PC * m: PC * m + NJ, :]                  # (C, NJ, B)
        s = s.reshape(C, NT, 2, B).transpose(2, 0, 1, 3)   # (h, C, NT, B)
        xpacks.append(np.ascontiguousarray(s.reshape(2 * C, NT, B)))

    # weight pack, bank-major: block (g, t) holds, for position p in
    # [lo, hi] and row (h*C + c), w[o, c, 128m+p, 2t+h-p] (zero when
    # 2t+h-p outside [0, 7)).
    wpacks = []
    wT = np.ascontiguousarray(weight.transpose(2, 3, 1, 0))  # (L, KW, C, O)
    for m in range(NCORES):
        wp = np.zeros((2, C, WCOLS), np.float32)
        for g in range(NBANK):
            for t, lo, hi in BANKS[g]:
                off = WOFF[(g, t)]
                for h in (0, 1):
                    j = 2 * t + h
                    for p in range(lo, hi + 1):
                        k = j - p
                        if 0 <= k < KW:
                            c0 = off + (p - lo) * O
                            wp[h, :, c0:c0 + O] = wT[PC * m + p, k]
        wpacks.append(np.ascontiguousarray(
            wp.reshape(2 * C, WCOLS)).astype(BF16))

    # bias pack: [1, PC*O + B]; trailing B ones are the stationary operand
    # of the per-bank bias-opener matmuls.
    bt = np.ascontiguousarray(bias.T)            # (L, O)
    bpacks = []
    for m in range(NCORES):
        bp = np.empty((1, PC * O + B), np.float32)
        bp[0, :PC * O] = bt[PC * m: PC * m + PC].reshape(-1)
        bp[0, PC * O:] = 1.0
        bpacks.append(bp.astype(BF16))
    return xpacks, wpacks, bpacks


_PROG = None


def _build_program():
    global _PROG
    if _PROG is not None:
        return _PROG

    import concourse.bacc as bacc
    import concourse.mybir as mybir
    import concourse.tile as tile

    F32 = mybir.dt.float32
    BF = mybir.dt.bfloat16

    nc = bacc.Bacc("TRN2", target_bir_lowering=False, debug=False,
                   num_devices=NCORES)
    x_d = nc.dram_tensor("xp", (2 * C, NT, B), BF, kind="ExternalInput")
    w_d = nc.dram_tensor("wp", (2 * C, WCOLS), BF, kind="ExternalInput")
    b_d = nc.dram_tensor("bp", (1, PC * O + B), BF, kind="ExternalInput")
    o_d = nc.dram_tensor("out", (B, PC * O), BF, kind="ExternalOutput")

    with tile.TileContext(nc) as tc:
        with (
            tc.tile_pool(name="xb", bufs=5) as xpool,
            tc.tile_pool(name="wb", bufs=10) as wpool,
            tc.tile_pool(name="cst", bufs=1) as cpool,
            tc.tile_pool(name="st", bufs=6) as spool,
            tc.tile_pool(name="ps", bufs=8, space="PSUM") as ppool,
        ):
            biast = cpool.tile([1, PC * O + B], BF)
            nc.sync.dma_start(biast[:], b_d[:])
            ones = biast[0:1, PC * O: PC * O + B]

            # Ring assignment: ALL weight chunks stream in bank order on the
            # scalar HWDGE ring, so arrival order == the PE's consumption
            # order (mixing rings reorders arrivals and leaves the PE
            # several banks behind — a cold multi-bank tail; a single ring
            # for everything costs ~10% peak bandwidth). x + bias ride the
            # sync ring and finish early; output stores go via the gpsimd
            # SWDGE path — a ring carrying loads AND stores crashes the
            # device, so HWDGE rings stay load-only.
            xtiles = {}
            wtiles = {}

            def _load_x(i):
                t0, n = XCHUNKS[i]
                xt = xpool.tile([2 * C, n * B], BF, tag="xt")
                nc.sync.dma_start(xt[:], x_d[:, t0:t0 + n, :])
                xtiles[i] = xt

            def _load_w(i):
                g0, n = WCHUNKS[i]
                c0, c1 = BANK_C0[g0], BANK_C0[g0 + n]
                wt = wpool.tile([2 * C, c1 - c0], BF, tag="wt")
                nc.scalar.dma_start(wt[:], w_d[:, c0:c1])
                wtiles[i] = wt

            _load_w(0)
            _load_x(0)
            _load_w(1)
            _load_x(1)
            _load_w(2)
            _load_x(2)
            _load_x(3)
            _load_x(4)
            for i in range(3, len(WCHUNKS)):
                _load_w(i)

            # Output staging: 4-bank chunks (4KB DMA rows) for the bulk,
            # then 2+1+1 so the final eviction->store chain after the last
            # matmul is short.
            STAGE_G0 = {0: 4, 4: 4, 8: 4, 12: 2, 14: 1, 15: 1}
            stage = None
            s_g0 = s_n = 0
            for g in range(NBANK):
                if g in STAGE_G0:
                    s_g0, s_n = g, STAGE_G0[g]
                    stage = spool.tile([B, s_n * BANKW], BF)
                ps = ppool.tile([B, BANKW], F32, tag="ps")
                # bias opener writes the full bank (start=True) so the
                # accumulating pieces land on uniformly-written psum.
                nc.tensor.matmul(
                    ps[:],
                    ones,
                    biast[0:1, BANKW * g: BANKW * (g + 1)],
                    start=True, stop=False,
                )
                wi, g0 = _wchunk_of(g)
                wt = wtiles[wi]
                wc0 = BANK_C0[g0]
                pieces = BANKS[g]
                for idx, (t, lo, hi) in enumerate(pieces):
                    xi, t0 = _xchunk_of(t)
                    xs = xtiles[xi][:, B * (t - t0): B * (t - t0 + 1)]
                    o0 = WOFF[(g, t)] - wc0
                    ws = wt[:, o0: o0 + (hi - lo + 1) * O]
                    nc.tensor.matmul(
                        ps[:, (lo - 8 * g) * O: (hi + 1 - 8 * g) * O],
                        xs,
                        ws,
                        start=False,
                        stop=(idx == len(pieces) - 1),
                    )
                sl = stage[:, BANKW * (g - s_g0): BANKW * (g - s_g0 + 1)]
                nc.vector.tensor_copy(sl, ps[:])
                if g == s_g0 + s_n - 1:
                    nc.gpsimd.dma_start(
                        o_d[:, BANKW * s_g0: BANKW * (s_g0 + s_n)], stage[:])

    nc.compile()
    _PROG = nc
    return nc


def _ensure_ntff_hook():
    """bass_utils' trace path imports antenv.axon_hooks, which this image
    lacks — if BASS_TRACE is set in the environment that import would crash.
    Install a minimal shim (ctypes into libaxon_pjrt.so; falls back to a
    no-hook stub that bass_utils handles by skipping the trace)."""
    import sys
    import types
    try:
        import antenv.axon_hooks  # noqa: F401
        return
    except ImportError:
        pass
    hook = None
    try:
        import contextlib
        import ctypes
        lib = ctypes.CDLL("/opt/axon/libaxon_pjrt.so")
        lib.axon_start_nrt_profile.argtypes = [
            ctypes.POINTER(ctypes.c_int64), ctypes.c_size_t]
        lib.axon_start_nrt_profile.restype = ctypes.c_int64
        lib.axon_stop_nrt_profile.argtypes = [ctypes.c_char_p]
        lib.axon_stop_nrt_profile.restype = ctypes.c_int64

        @contextlib.contextmanager
        def _hook(output_dir, device_ids):
            import jax
            jax.devices()
            if device_ids:
                ids = (ctypes.c_int64 * len(device_ids))(*device_ids)
                rc = lib.axon_start_nrt_profile(ids, len(device_ids))
            else:
                rc = lib.axon_start_nrt_profile(None, 0)
            if rc != 0:
                raise RuntimeError(f"axon_start_nrt_profile rc={rc}")
            try:
                yield
            finally:
                lib.axon_stop_nrt_profile(str(output_dir).encode())

        hook = _hook
    except Exception:
        hook = None
    mod = types.ModuleType("antenv.axon_hooks")
    mod.get_axon_ntff_profile_hook = lambda: hook
    mod.set_axon_ntff_profile_hook = lambda h: None
    try:
        import antenv
        antenv.axon_hooks = mod
    except ImportError:
        pass
    sys.modules["antenv.axon_hooks"] = mod


def _run(x, weight, bias, trace=False, tmpdir=None):
    from concourse.bass_utils import run_bass_kernel_spmd
    _ensure_ntff_hook()

    x = np.asarray(x, dtype=np.float32)
    weight = np.asarray(weight, dtype=np.float32)
    bias = np.asarray(bias, dtype=np.float32)
    xpacks, wpacks, bpacks = _pack_inputs(x, weight, bias)
    nc = _build_program()
    in_maps = [{"xp": xpacks[m], "wp": wpacks[m], "bp": bpacks[m]}
               for m in range(NCORES)]
    res = run_bass_kernel_spmd(nc, in_maps, list(range(NCORES)), trace=trace,
                               tmpdir=tmpdir)
    outs = [r["out"].astype(np.float32).reshape(B, PC, O).transpose(0, 2, 1)
            for r in res.results]
    full = np.ascontiguousarray(np.concatenate(outs, axis=2))
    return full, res


def kernel(x, weight, bias):
    out, _ = _run(x, weight, bias, trace=False)
    return out


# revision 24
# speedup vs baseline: 1.1189x; 1.1189x over previous
"""LocallyConnected1d (untied-weight conv1d) on 8 Trainium2 NeuronCores.

Problem (hardcoded):
    x:      (B=128, C=64, L=1024) f32
    weight: (O=64, C=64, P=1024, K=7) f32   (untied per output position)
    bias:   (O=64, P=1024) f32
    out:    (B=128, O=64, P=1024) f32
    out[b,o,p] = sum_{c,k} xpad[b,c,p+k] * w[o,c,p,k] + bias[o,p]  (pad=3)

Sharding: sequence-parallel over P — core m owns positions [128m, 128m+128).
Each weight element is used exactly once, so this minimizes HBM traffic
(weight shard dominates); x, weight and the output are cast to bf16
(PSUM accumulation stays fp32), halving the dominant DMA traffic.

Per-core algorithm (pair scheme): input columns are processed in PAIRS
(j=2t, 2t+1) stacked into the full 128-partition contraction dim
[(h, c) = 2*64]. Pair t's stationary operand is the x pair-column
[128, b=128]; for each PSUM bank g (8 output positions), the 7 pairs
t=4g..4g+6 contribute moving blocks of untied weights [128, width*64]
(width = overlap of the pair's 8-position span with the bank, summing to
32 position-instances = 2048 moving columns per bank — half the moving
columns of a 64-row scheme, which matters because the HAM clock gate
keeps the PE at 1.2GHz in this DMA-bound regime: PE work must fit under
the DMA time even when cold). Positions covered by only one column of
the pair get 64 zero rows in the weight pack (~12% pad bytes). Bias is
added by opening each bank's accumulation group with a K=1 ones-x-bias
matmul that writes the full bank.
"""

import numpy as np
import ml_dtypes

BF16 = ml_dtypes.bfloat16

B = 128
C = 64
O = 64
L = 1024
KW = 7
PAD = 3
NCORES = 8
PC = L // NCORES          # positions per core = 128
NJ = PC + 2 * PAD         # input columns per core incl halo = 134
NT = NJ // 2              # column pairs = 67
NBANK = PC // 8           # psum banks of 8 positions = 16
BANKW = 8 * O             # psum bank free width = 512 f32


def _bank_pairs(g):
    """Pairs contributing to bank g: (t, lo, hi) position ranges."""
    out = []
    for t in range(NT):
        lo = max(8 * g, 2 * t - (KW - 1), 0)
        hi = min(8 * g + 7, 2 * t + 1, PC - 1)
        if lo <= hi:
            out.append((t, lo, hi))
    return out


BANKS = [_bank_pairs(g) for g in range(NBANK)]
WOFF = {}                 # weight-pack column offset of block (g, t)
_off = 0
for _g in range(NBANK):
    for _t, _lo, _hi in BANKS[_g]:
        WOFF[(_g, _t)] = _off
        _off += (_hi - _lo + 1) * O
WCOLS = _off              # 32768
BANK_C0 = [WOFF[(g, BANKS[g][0][0])] for g in range(NBANK)] + [WCOLS]

# x DMA chunks in pairs; weight DMA chunks in banks (small first chunk so
# compute starts early, ~1MB mid-kernel, small last chunk for a short tail).
XCHUNKS = [(0, 17), (17, 17), (34, 17), (51, 16)]
WCHUNKS = []
_g = 0
for _n in [1] + [2] * 7 + [1]:
    WCHUNKS.append((_g, _n))
    _g += _n
assert _g == NBANK


def _xchunk_of(t):
    for i, (t0, n) in enumerate(XCHUNKS):
        if t0 <= t < t0 + n:
            return i, t0
    raise AssertionError


def _wchunk_of(g):
    for i, (g0, n) in enumerate(WCHUNKS):
        if g0 <= g < g0 + n:
            return i, g0
    raise AssertionError


def _pack_inputs(x, weight, bias):
    """Host-side relayout into DMA/matmul-friendly per-core arrays (bf16)."""
    xp = np.zeros((B, C, L + 2 * PAD), np.float32)
    xp[:, :, PAD:PAD + L] = x
    # (C, 1030, B): column-major access per (c, j)
    xt = np.ascontiguousarray(xp.transpose(1, 2, 0)).astype(BF16)

    xpacks = []
    for m in range(NCORES):
        s = xt[:, PC * m: PC * m + NJ, :]                  # (C, NJ, B)
        s = s.reshape(C, NT, 2, B).transpose(2, 0, 1, 3)   # (h, C, NT, B)
        xpacks.append(np.ascontiguousarray(s.reshape(2 * C, NT, B)))

    # weight pack, bank-major: block (g, t) holds, for position p in
    # [lo, hi] and row (h*C + c), w[o, c, 128m+p, 2t+h-p] (zero when
    # 2t+h-p outside [0, 7)).
    wpacks = []
    wT = np.ascontiguousarray(weight.transpose(2, 3, 1, 0))  # (L, KW, C, O)
    for m in range(NCORES):
        wp = np.zeros((2, C, WCOLS), np.float32)
        for g in range(NBANK):
            for t, lo, hi in BANKS[g]:
                off = WOFF[(g, t)]
                for h in (0, 1):
                    j = 2 * t + h
                    for p in range(lo, hi + 1):
                        k = j - p
                        if 0 <= k < KW:
                            c0 = off + (p - lo) * O
                            wp[h, :, c0:c0 + O] = wT[PC * m + p, k]
        wpacks.append(np.ascontiguousarray(
            wp.reshape(2 * C, WCOLS)).astype(BF16))

    # bias pack: [1, PC*O + B]; trailing B ones are the stationary operand
    # of the per-bank bias-opener matmuls.
    bt = np.ascontiguousarray(bias.T)            # (L, O)
    bpacks = []
    for m in range(NCORES):
        bp = np.empty((1, PC * O + B), np.float32)
        bp[0, :PC * O] = bt[PC * m: PC * m + PC].reshape(-1)
        bp[0, PC * O:] = 1.0
        bpacks.append(bp.astype(BF16))
    return xpacks, wpacks, bpacks


_PROG = None


def _build_program():
    global _PROG
    if _PROG is not None:
        return _PROG

    import concourse.bacc as bacc
    import concourse.mybir as mybir
    import concourse.tile as tile

    F32 = mybir.dt.float32
    BF = mybir.dt.bfloat16

    nc = bacc.Bacc("TRN2", target_bir_lowering=False, debug=False,
                   num_devices=NCORES)
    x_d = nc.dram_tensor("xp", (2 * C, NT, B), BF, kind="ExternalInput")
    w_d = nc.dram_tensor("wp", (2 * C, WCOLS), BF, kind="ExternalInput")
    b_d = nc.dram_tensor("bp", (1, PC * O + B), BF, kind="ExternalInput")
    o_d = nc.dram_tensor("out", (B, PC * O), BF, kind="ExternalOutput")

    with tile.TileContext(nc) as tc:
        with (
            tc.tile_pool(name="xb", bufs=4) as xpool,
            tc.tile_pool(name="wb", bufs=10) as wpool,
            tc.tile_pool(name="cst", bufs=1) as cpool,
            tc.tile_pool(name="st", bufs=6) as spool,
            tc.tile_pool(name="ps", bufs=8, space="PSUM") as ppool,
        ):
            biast = cpool.tile([1, PC * O + B], BF)
            nc.sync.dma_start(biast[:], b_d[:])
            ones = biast[0:1, PC * O: PC * O + B]

            # Ring assignment: ALL weight chunks stream in bank order on the
            # scalar HWDGE ring, so arrival order == the PE's consumption
            # order (mixing rings reorders arrivals and leaves the PE
            # several banks behind — a cold multi-bank tail; a single ring
            # for everything costs ~10% peak bandwidth). x + bias ride the
            # sync ring and finish early; output stores go via the gpsimd
            # SWDGE path — a ring carrying loads AND stores crashes the
            # device, so HWDGE rings stay load-only.
            xtiles = {}
            wtiles = {}

            def _load_x(i):
                t0, n = XCHUNKS[i]
                xt = xpool.tile([2 * C, n * B], BF, tag="xt")
                nc.sync.dma_start(xt[:], x_d[:, t0:t0 + n, :])
                xtiles[i] = xt

            def _load_w(i):
                g0, n = WCHUNKS[i]
                c0, c1 = BANK_C0[g0], BANK_C0[g0 + n]
                wt = wpool.tile([2 * C, c1 - c0], BF, tag="wt")
                nc.scalar.dma_start(wt[:], w_d[:, c0:c1])
                wtiles[i] = wt

            _load_w(0)
            _load_x(0)
            _load_w(1)
            _load_x(1)
            _load_w(2)
            _load_x(2)
            _load_x(3)
            for i in range(3, len(WCHUNKS)):
                _load_w(i)

            # Output staging: 4-bank chunks (4KB DMA rows) for the bulk,
            # then 2+1+1 so the final eviction->store chain after the last
            # matmul is short.
            STAGE_G0 = {0: 4, 4: 4, 8: 4, 12: 2, 14: 1, 15: 1}
            stage = None
            s_g0 = s_n = 0
            for g in range(NBANK):
                if g in STAGE_G0:
                    s_g0, s_n = g, STAGE_G0[g]
                    stage = spool.tile([B, s_n * BANKW], BF)
                ps = ppool.tile([B, BANKW], F32, tag="ps")
                # bias opener writes the full bank (start=True) so the
                # accumulating pieces land on uniformly-written psum.
                nc.tensor.matmul(
                    ps[:],
                    ones,
                    biast[0:1, BANKW * g: BANKW * (g + 1)],
                    start=True, stop=False,
                )
                wi, g0 = _wchunk_of(g)
                wt = wtiles[wi]
                wc0 = BANK_C0[g0]
                pieces = BANKS[g]
                for idx, (t, lo, hi) in enumerate(pieces):
                    xi, t0 = _xchunk_of(t)
                    xs = xtiles[xi][:, B * (t - t0): B * (t - t0 + 1)]
                    o0 = WOFF[(g, t)] - wc0
                    ws = wt[:, o0: o0 + (hi - lo + 1) * O]
                    nc.tensor.matmul(
                        ps[:, (lo - 8 * g) * O: (hi + 1 - 8 * g) * O],
                        xs,
                        ws,
                        start=False,
                        stop=(idx == len(pieces) - 1),
                    )
                sl = stage[:, BANKW * (g - s_g0): BANKW * (g - s_g0 + 1)]
                nc.vector.tensor_copy(sl, ps[:])
                if g == s_g0 + s_n - 1:
                    nc.gpsimd.dma_start(
                        o_d[:, BANKW * s_g0: BANKW * (s_g0 + s_n)], stage[:])

    nc.compile()
    _PROG = nc
    return nc


def _ensure_ntff_hook():
    """bass_utils' trace path imports antenv.axon_hooks, which this image
    lacks — if BASS_TRACE is set in the environment that import would crash.
    Install a minimal shim (ctypes into libaxon_pjrt.so; falls back to a
    no-hook stub that bass_utils handles by skipping the trace)."""
    import sys
    import types
    try:
        import antenv.axon_hooks  # noqa: F401
        return
    except ImportError:
        pass
    hook = None
    try:
        import contextlib
        import ctypes
        lib = ctypes.CDLL("/opt/axon/libaxon_pjrt.so")
        lib.axon_start_nrt_profile.argtypes = [
            ctypes.POINTER(ctypes.c_int64), ctypes.c_size_t]
        lib.axon_start_nrt_profile.restype = ctypes.c_int64
        lib.axon_stop_nrt_profile.argtypes = [ctypes.c_char_p]
        lib.axon_stop_nrt_profile.restype = ctypes.c_int64

        @contextlib.contextmanager
        def _hook(output_dir, device_ids):
            import jax
            jax.devices()
            if device_ids:
                ids = (ctypes.c_int64 * len(device_ids))(*device_ids)
                rc = lib.axon_start_nrt_profile(ids, len(device_ids))
            else:
                rc = lib.axon_start_nrt_profile(None, 0)
            if rc != 0:
                raise RuntimeError(f"axon_start_nrt_profile rc={rc}")
            try:
                yield
            finally:
                lib.axon_stop_nrt_profile(str(output_dir).encode())

        hook = _hook
    except Exception:
        hook = None
    mod = types.ModuleType("antenv.axon_hooks")
    mod.get_axon_ntff_profile_hook = lambda: hook
    mod.set_axon_ntff_profile_hook = lambda h: None
    try:
        import antenv
        antenv.axon_hooks = mod
    except ImportError:
        pass
    sys.modules["antenv.axon_hooks"] = mod


def _run(x, weight, bias, trace=False, tmpdir=None):
    from concourse.bass_utils import run_bass_kernel_spmd
    _ensure_ntff_hook()

    x = np.asarray(x, dtype=np.float32)
    weight = np.asarray(weight, dtype=np.float32)
    bias = np.asarray(bias, dtype=np.float32)
    xpacks, wpacks, bpacks = _pack_inputs(x, weight, bias)
    nc = _build_program()
    in_maps = [{"xp": xpacks[m], "wp": wpacks[m], "bp": bpacks[m]}
               for m in range(NCORES)]
    res = run_bass_kernel_spmd(nc, in_maps, list(range(NCORES)), trace=trace,
                               tmpdir=tmpdir)
    outs = [r["out"].astype(np.float32).reshape(B, PC, O).transpose(0, 2, 1)
            for r in res.results]
    full = np.ascontiguousarray(np.concatenate(outs, axis=2))
    return full, res


def kernel(x, weight, bias):
    out, _ = _run(x, weight, bias, trace=False)
    return out
